# revision 22
# baseline (speedup 1.0000x reference)
"""CrossViewAttention3D Trainium2 kernel.

B=1, C=512, T=4, H=32, W=32 -> N=4096 tokens, 8 heads x head_dim 64.
Head-parallel across 8 NeuronCores: core h computes q/k/v projections for
its head, fused flash-style attention (S^T tiles -> exp on ACT -> AV
accumulate in PSUM, softmax denominator via a ones-column appended to
v^T), then the Wo column-slice partial out-projection.  Host sums the 8
partials and adds the output bias.

Matmuls run in fp16 (1 cycle/col on the PE vs 2 for fp32r, ~5e-4 rel
precision).  The K=64 S^T matmuls are row-packed in pairs via
tile_position (0,0)/(64,0); q and k are duplicated across partitions
0-63 / 64-127 (host duplicates the weight columns, so the projection
matmuls produce both copies for free).

Self-contained: hardcodes all shapes; needs numpy + the installed
concourse/bass stack (axon-attached TRN2 cores via jax).
"""
import numpy as np

import concourse.tile as tile
from concourse import bacc, mybir
from concourse.bass_utils import run_bass_kernel_spmd
from concourse.masks import make_identity

f32 = mybir.dt.float32
MMDT = mybir.dt.float16     # matmul operand dtype

B, C, T, H, W = 1, 512, 4, 32, 32
NHEADS = 8
D = C // NHEADS          # 64 head dim
N = T * H * W            # 4096 tokens
P = 128                  # partitions
NT = 512                 # n-tile (matmul moving dim)
NTILES = N // NT         # 8
CCH = C // P             # 4 c-chunks
MCH = N // P             # 32 m-chunks
NPAIR = MCH // 2         # 16 m-chunk pairs (row-packed S matmuls)
SCALE = float(D) ** -0.5  # 0.125

_EXP = mybir.ActivationFunctionType.Exp


def _build():
    nc = bacc.Bacc(None, target_bir_lowering=False, debug=False)
    xv = nc.dram_tensor("xv", [C, N], f32, kind="ExternalInput")
    xr = nc.dram_tensor("xr", [C, N], f32, kind="ExternalInput")
    # wq/wk carry the head weight columns duplicated (host sends [C, 2D])
    wq = nc.dram_tensor("wq", [C, 2 * D], f32, kind="ExternalInput")
    wk = nc.dram_tensor("wk", [C, 2 * D], f32, kind="ExternalInput")
    wv = nc.dram_tensor("wv", [C, D], f32, kind="ExternalInput")
    bq = nc.dram_tensor("bq", [2 * D, 1], f32, kind="ExternalInput")
    bk = nc.dram_tensor("bk", [2 * D, 1], f32, kind="ExternalInput")
    bv = nc.dram_tensor("bv", [D, 1], f32, kind="ExternalInput")
    wo = nc.dram_tensor("wo", [D, C], f32, kind="ExternalInput")
    out = nc.dram_tensor("out", [C, N], f32, kind="ExternalOutput")

    xv_r = xv.rearrange("(o p) n -> p o n", p=P)
    xr_r = xr.rearrange("(o p) n -> p o n", p=P)

    with tile.TileContext(nc) as tc:
        with (
            tc.tile_pool(name="const", bufs=1) as const,
            tc.tile_pool(name="persist", bufs=1) as persist,
            tc.tile_pool(name="xload", bufs=4) as xload,
            tc.tile_pool(name="ptile", bufs=4) as ptile,
            tc.tile_pool(name="stage", bufs=3) as stage,
        ):
            # ---- weights / biases / identity ----
            wq_sb = const.tile([P, CCH, 2 * D], MMDT, tag="wq")
            wk_sb = const.tile([P, CCH, 2 * D], MMDT, tag="wk")
            wv_sb = const.tile([P, CCH, D], MMDT, tag="wv")
            nc.gpsimd.dma_start(wq_sb[:], wq.rearrange("(o p) m -> p o m", p=P))
            nc.gpsimd.dma_start(wk_sb[:], wk.rearrange("(o p) m -> p o m", p=P))
            nc.gpsimd.dma_start(wv_sb[:], wv.rearrange("(o p) m -> p o m", p=P))
            wo_sb = const.tile([D, C], MMDT, tag="wo")
            nc.gpsimd.dma_start(wo_sb[:], wo[:])
            bq_sb = const.tile([2 * D, 1], f32, tag="bq")
            bk_sb = const.tile([2 * D, 1], f32, tag="bk")
            bv_sb = const.tile([D, 1], f32, tag="bv")
            nc.sync.dma_start(bq_sb[:], bq[:])
            nc.sync.dma_start(bk_sb[:], bk[:])
            nc.sync.dma_start(bv_sb[:], bv[:])

            ident = const.tile([D, D], MMDT, tag="ident")
            make_identity(nc, ident[:])
            # ---- persistent activations ----
            q_sb = persist.tile([P, N], MMDT, tag="q")    # rows 64:128 dup
            k_sb = persist.tile([P, N], MMDT, tag="k")
            v_sb = persist.tile([D, N], MMDT, tag="v")
            v1t = persist.tile([P, MCH, D + 1], MMDT, tag="v1t")
            ones_sb = const.tile([P, MCH], f32, tag="ones")
            nc.vector.memset(ones_sb[:], 1.0)
            nc.vector.tensor_copy(v1t[:, :, D], ones_sb[:])

            # ---- PSUM pools (flat, 8 banks total) ----
            with (
                tc.tile_pool(name="ps_s", bufs=2, space="PSUM") as ps_s,
                tc.tile_pool(name="ps_o", bufs=2, space="PSUM") as ps_o,
                tc.tile_pool(name="ps_op", bufs=2, space="PSUM") as ps_op,
            ):
                # ---- helpers ----
                def load_x(dram_r, nt, tag):
                    # per-c-chunk DMAs + casts so projection matmuls start on
                    # the first 256KB instead of the full tile
                    ns = slice(nt * NT, (nt + 1) * NT)
                    raw = xload.tile([P, CCH, NT], f32, tag=tag + "r",
                                     name=f"{tag}r_{nt}")
                    x16 = xload.tile([P, CCH, NT], MMDT, tag=tag,
                                     name=f"{tag}_{nt}")
                    for cc in range(CCH):
                        nc.sync.dma_start(raw[:, cc], dram_r[:, cc, ns])
                        nc.vector.tensor_copy(x16[:, cc], raw[:, cc])
                    return x16

                def proj(dst, w_sb, b_sb, x16, nt, rows):
                    ns = slice(nt * NT, (nt + 1) * NT)
                    ps = ps_op.tile([P, NT], f32, tag="op", name=f"pj_{nt}")
                    for cc in range(CCH):
                        nc.tensor.matmul(ps[:rows], w_sb[:, cc], x16[:, cc],
                                         start=(cc == 0), stop=(cc == CCH - 1))
                    nc.vector.tensor_add(dst[:, ns], ps[:rows],
                                         b_sb[:, 0:1].to_broadcast([rows, NT]))

                def q_proj(nt):
                    x16 = load_x(xv_r, nt, "xv")
                    proj(q_sb, wq_sb, bq_sb, x16, nt, P)

                o_tiles = {}
                p_map = {}

                def emit_s(nt, j):
                    ns = slice(nt * NT, (nt + 1) * NT)
                    mca, mcb = 2 * j, 2 * j + 1
                    s_ps = ps_s.tile([P, 2, NT], f32, tag="s",
                                     name=f"s_{nt}_{j}")
                    nc.tensor.matmul(
                        s_ps[:, 0], k_sb[0:D, mca * P:(mca + 1) * P],
                        q_sb[0:D, ns], start=True, stop=True,
                        tile_position=(0, 0))
                    nc.tensor.matmul(
                        s_ps[:, 1], k_sb[D:P, mcb * P:(mcb + 1) * P],
                        q_sb[D:P, ns], start=True, stop=True,
                        tile_position=(64, 0))
                    p_t = ptile.tile([P, 2, NT], MMDT, tag="p",
                                     name=f"p_{nt}_{j}")
                    nc.scalar.activation(p_t[:], s_ps[:], _EXP, scale=SCALE)
                    p_map[(nt, j)] = p_t

                def emit_av(nt, j):
                    o_ps = o_tiles[nt]
                    p_t = p_map.pop((nt, j))
                    mca, mcb = 2 * j, 2 * j + 1
                    nc.tensor.matmul(o_ps[:D + 1], v1t[:, mca], p_t[:, 0],
                                     start=(j == 0), stop=False)
                    nc.tensor.matmul(o_ps[:D + 1], v1t[:, mcb], p_t[:, 1],
                                     start=False, stop=(j == NPAIR - 1))

                from collections import deque
                SKEW = 3
                av_q = deque()

                def push_s(nt, j):
                    emit_s(nt, j)
                    av_q.append((nt, j))
                    while len(av_q) > SKEW:
                        emit_av(*av_q.popleft())

                def drain_avs():
                    while av_q:
                        emit_av(*av_q.popleft())

                def epilogue_head(nt):
                    # frees the O accumulator quickly; normalization by the
                    # softmax denominator commutes with the Wo matmul, so the
                    # out-projection consumes UNNORMALIZED O and the divide
                    # happens on the projected tiles in epilogue_tail.
                    o_ps = o_tiles.pop(nt)
                    obar16 = stage.tile([D, NT], MMDT, tag="obar")
                    nc.vector.tensor_copy(obar16[:], o_ps[:D])
                    den = stage.tile([1, NT], f32, tag="den")
                    nc.vector.tensor_copy(den[:], o_ps[D:D + 1])
                    rec = stage.tile([1, NT], f32, tag="rec")
                    rscr = stage.tile([1, NT], f32, tag="rscr")
                    nc.vector.reciprocal_approx_accurate(rec[:], den[:],
                                                         rscr[:])
                    rb = stage.tile([P, NT], f32, tag="rb")
                    nc.gpsimd.partition_broadcast(rb[:], rec[:])
                    return obar16, rb

                def epilogue_tail(nt, obar16, rb):
                    ns = slice(nt * NT, (nt + 1) * NT)
                    for cc in range(CCH):
                        op_ps = ps_op.tile([P, NT], f32, tag="op",
                                           name=f"opj_{nt}_{cc}")
                        nc.tensor.matmul(op_ps[:],
                                         wo_sb[:, cc * P:(cc + 1) * P],
                                         obar16[:], start=True, stop=True)
                        ot = stage.tile([P, NT], f32, tag="ot")
                        nc.vector.tensor_mul(ot[:], op_ps[:], rb[:])
                        nc.sync.dma_start(out[cc * P:(cc + 1) * P, ns], ot[:])

                # ---- interleaved prologue + passes 0 and 1 ----
                # group g: load xr tile g, project k/v, transpose v chunks;
                # pass-0/1 S-pairs slot in behind the k/v1t chunks they need
                # so ACT starts filling while the prologue is still streaming.
                # AV matmuls trail their S-pair by SKEW slots globally, so
                # independent S work always sits between dependent AVs in the
                # PE FIFO (incl. across pass boundaries).
                o_tiles[0] = ps_o.tile([P, NT], f32, tag="o", name="o_0")
                o_tiles[1] = ps_o.tile([P, NT], f32, tag="o", name="o_1")
                for g in range(NTILES):
                    x16 = load_x(xr_r, g, "xr")
                    proj(k_sb, wk_sb, bk_sb, x16, g, P)
                    proj(v_sb, wv_sb, bv_sb, x16, g, D)
                    for mc in range(4 * g, 4 * g + 4):
                        vt_ps = ps_op.tile([P, D], MMDT, tag="op",
                                           name=f"vt_{mc}")
                        nc.tensor.transpose(
                            vt_ps[:], v_sb[:, mc * P:(mc + 1) * P], ident[:])
                        nc.vector.tensor_copy(v1t[:, mc, 0:D], vt_ps[:])
                    if g == 0:
                        q_proj(0)
                        push_s(0, 0)
                        push_s(0, 1)
                    elif g == 1:
                        push_s(0, 2)
                        push_s(0, 3)
                        q_proj(1)
                    else:
                        push_s(0, 2 * g)
                        push_s(0, 2 * g + 1)
                        push_s(1, 2 * (g - 2))
                        push_s(1, 2 * (g - 2) + 1)
                q_proj(2)
                pendings = deque()
                for j in range(2 * (NTILES - 2), NPAIR):
                    push_s(1, j)
                    if (0, NPAIR - 1) not in p_map and (0, NPAIR - 1) not in av_q \
                            and 0 in o_tiles and not pendings:
                        pendings.append([0, *epilogue_head(0)])

                # ---- remaining passes ----
                for nt in range(2, NTILES):
                    o_tiles[nt] = ps_o.tile([P, NT], f32, tag="o",
                                            name=f"o_{nt}")
                    for j in range(NPAIR):
                        push_s(nt, j)
                        prev_done = ((nt - 1) in o_tiles
                                     and (nt - 1, NPAIR - 1) not in av_q
                                     and (nt - 1, NPAIR - 1) not in p_map)
                        if prev_done:
                            pendings.append([nt - 1, *epilogue_head(nt - 1)])
                        if j == 4 and len(pendings) >= 2:
                            epilogue_tail(*pendings.popleft())
                        if j == 12 and pendings:
                            epilogue_tail(*pendings.popleft())
                        if j == 8 and nt + 1 < NTILES:
                            q_proj(nt + 1)
                drain_avs()
                pendings.append([NTILES - 1, *epilogue_head(NTILES - 1)])
                while pendings:
                    epilogue_tail(*pendings.popleft())
    nc.compile()
    return nc


_cached_nc = None


def _get_nc():
    global _cached_nc
    if _cached_nc is None:
        _cached_nc = _build()
    return _cached_nc


def _make_in_maps(inp):
    xv = np.ascontiguousarray(inp["video_feat"].reshape(C, N), dtype=np.float32)
    xr = np.ascontiguousarray(inp["ref_feat"].reshape(C, N), dtype=np.float32)

    def dupc(a):  # duplicate columns: [C, D] -> [C, 2D]
        return np.ascontiguousarray(np.concatenate([a, a], axis=1),
                                    dtype=np.float32)

    in_maps = []
    for h in range(NHEADS):
        sl = slice(h * D, (h + 1) * D)
        wq_t = inp["Wq"][sl].T
        wk_t = inp["Wk"][sl].T
        in_maps.append({
            "xv": xv,
            "xr": xr,
            "wq": dupc(wq_t),
            "wk": dupc(wk_t),
            "wv": np.ascontiguousarray(inp["Wv"][sl].T, dtype=np.float32),
            "bq": np.ascontiguousarray(
                np.tile(inp["bq"][sl], 2).reshape(2 * D, 1), dtype=np.float32),
            "bk": np.ascontiguousarray(
                np.tile(inp["bk"][sl], 2).reshape(2 * D, 1), dtype=np.float32),
            "bv": np.ascontiguousarray(
                inp["bv"][sl].reshape(D, 1), dtype=np.float32),
            "wo": np.ascontiguousarray(inp["Wo"][:, sl].T, dtype=np.float32),
        })
    return in_maps


def run(inputs, **spmd_kwargs):
    """Run the kernel; returns (full_output, BassKernelResults)."""
    inp = {k: np.asarray(v) for k, v in inputs.items()}
    nc = _get_nc()
    res = run_bass_kernel_spmd(nc, _make_in_maps(inp),
                               list(range(NHEADS)), **spmd_kwargs)
    total = res.results[0]["out"].astype(np.float32).copy()
    for r in res.results[1:]:
        total += r["out"]
    total += np.asarray(inp["bo"], dtype=np.float32)[:, None]
    return total.reshape(B, C, T, H, W), res


def kernel(**inputs):
    out, _ = run(inputs)
    return out
